# revision 6
# baseline (speedup 1.0000x reference)
"""2-layer GAT on 8 Trainium2 NeuronCores (Bass/Tile).

Sharding: nodes sorted by (in-degree, low-half-src count), snake-dealt
across 8 cores (6250 -> padded 6272/core), tiled 128/tile (49 tiles);
partition j of tile t owns one dst node, its in-edges occupy slots
(chunk c, partition j). Node table in HBM (row = [f 256 | el 8 | er 8 |
pad] f32, 1280B) is built by the projection matmul x @ [W1|W1.al1|W1.ar1].
Edge rows are fetched with the 16-lane dma_gather custom DMA; its int16
index limit (<32768 rows) is beaten by splitting each tile's slots into
two grids gathered from two table views (rows 0..25088 and 25089..50177),
with per-node slot counts equalized by the (deg, a)-sort. alpha =
exp(leaky_relu(el[src]+er[dst])) (no max-subtraction: logits are small);
alpha overwrites the el column so one identity-matmul per chunk
accumulates [sum(alpha*f)|sum(alpha)] in PSUM; divide, ELU. Layer-2
projection h1 @ [W2|wl2|wr2] per tile; the host assembles the full
256B-row layer-2 table for launch 2 (same grids). Padding slots point at
sentinel rows (f=0, el=-300 -> alpha ~= 0).
"""
import sys

sys.path.insert(0, "/opt/trn_rl_repo")

import numpy as np

import concourse.bass as bass
import concourse.bacc as bacc
import concourse.tile as tile
from concourse import mybir
from concourse.bass_utils import run_bass_kernel_spmd

N = 50000
E = 800000
P = 128
NCORES = 8
TILES = 49
NPC = TILES * P                  # 6272
NPAD = NCORES * NPC              # 50176
GBLOCKS = NPAD // P              # 392
SPLIT_ROW = 25088                # sentinel-A row; B view starts at 25089
NTAB = NPAD + 2                  # 50178
SENT_A = SPLIT_ROW
SENT_B = NTAB - 1
ROW1 = 320                       # [f 256 | el 8 | er 8 | pad 48]  (1280B)
ROW2 = 64                        # [f2 32 | el2 1 | er2 1 | pad]   (256B)
H1, D1 = 8, 32
NEG_SLOPE = 0.2
SENT_EL = -300.0
F32 = mybir.dt.float32
I16 = mybir.dt.int16
I32 = mybir.dt.int32


def _new_row(r):
    return r + (r >= SPLIT_ROW)


def _ap(t, off, dims):
    s = t[:] if not isinstance(t, bass.AP) else t
    return bass.AP(tensor=s.tensor, offset=s.offset + off, ap=[s.ap[0]] + dims)


# ----------------------------------------------------------------------------
# host preprocessing
# ----------------------------------------------------------------------------

def _prep(src, dst):
    deg = np.bincount(dst, minlength=N)
    # a = number of in-edges whose src lands in the low table half.
    # Low half = new ids < SPLIT_ROW; new ids depend on this sort, so
    # bootstrap: sort by degree first to fix newid, then a is known only
    # after... break the cycle by sorting on (deg, a_orig) where a_orig is
    # computed from the degree-only assignment.
    order0 = np.argsort(-deg, kind="stable")
    pat = np.concatenate([np.arange(NCORES), np.arange(NCORES - 1, -1, -1)])
    core_of_pos = pat[np.arange(N) % (2 * NCORES)]
    newid0 = np.empty(N, np.int64)
    for c in range(NCORES):
        nodes_c = order0[core_of_pos == c]
        newid0[nodes_c] = c * NPC + np.arange(len(nodes_c))
    low0 = (newid0[src] < SPLIT_ROW).astype(np.int64)
    a_of = np.bincount(dst, weights=low0, minlength=N).astype(np.int64)

    # final order: degree desc, then a desc (keeps per-tile (deg, a) tight)
    order = np.lexsort((-a_of, -deg))
    newid = np.empty(N, np.int64)
    for c in range(NCORES):
        nodes_c = order[core_of_pos == c]
        newid[nodes_c] = c * NPC + np.arange(len(nodes_c))

    nd = newid[dst]
    ns = newid[src]
    low = ns < SPLIT_ROW

    # per-dst counts of low/high srcs
    aA = np.bincount(nd, weights=low, minlength=NPAD).astype(np.int64)
    aB = np.bincount(nd, weights=~low, minlength=NPAD).astype(np.int64)
    TA = aA.reshape(NCORES, TILES, P).max(axis=(0, 2)).astype(np.int64)
    TB = aB.reshape(NCORES, TILES, P).max(axis=(0, 2)).astype(np.int64)
    TB = np.maximum(TB, 1)       # ensure >=1 chunk per tile overall

    # slot assignment: sort edges by (nd, high?) so each dst's A-edges come
    # first; slot k within the group.
    keys = nd * 2 + (~low)
    o = np.argsort(keys, kind="stable")
    nd_s, ns_s, low_s = nd[o], ns[o], low[o]
    ks = np.zeros(E, np.int64)
    kk = np.argsort(o, kind="stable")  # not needed; compute group ranks:
    first = np.searchsorted(keys[o], np.arange(2 * NPAD), side="left")
    ks = np.arange(E) - first[keys[o]]

    offA = np.concatenate([[0], np.cumsum(TA)])
    offB = np.concatenate([[0], np.cumsum(TB)])
    TSA, TSB = int(offA[-1]), int(offB[-1])

    c_s = nd_s // NPC
    t_s = (nd_s % NPC) // P
    j_s = nd_s % P

    # packed int16 index streams, [NCORES, 16, 8*TS]
    idxA = np.full((NCORES, 16, 8 * TSA), SENT_A, np.int16)
    idxB = np.full((NCORES, 16, 8 * TSB), SENT_B - (SPLIT_ROW + 1), np.int16)
    # flat slot id within the per-tile grid: i = c*128 + j  (c = chunk)
    iA = (offA[t_s] + ks) * P + j_s
    iB = (offB[t_s] + ks) * P + j_s
    vA = ns_s                        # A view row == new id (< SPLIT_ROW)
    vB = ns_s + 1 - (SPLIT_ROW + 1)  # B view: row = ns+1, rebased
    m = low_s.astype(bool)
    idxA[c_s[m], iA[m] % 16, iA[m] // 16] = vA[m].astype(np.int16)
    idxB[c_s[~m], iB[~m] % 16, iB[~m] // 16] = vB[~m].astype(np.int16)
    idxA = np.tile(idxA, (1, 8, 1))
    idxB = np.tile(idxB, (1, 8, 1))

    # per-core table row of own node (t, j) for the er fetch
    own = np.arange(NPAD).reshape(NCORES, TILES, P)
    ert = _new_row(own).transpose(0, 2, 1).astype(np.int32)  # [NC, P, TILES]

    return {"newid": newid, "TA": TA, "TB": TB, "idxA": idxA, "idxB": idxB,
            "ert": np.ascontiguousarray(ert)}


# ----------------------------------------------------------------------------
# launch builders
# ----------------------------------------------------------------------------

def _edge_tile(nc, pool, small, psum, tc, *, t, TA, TB, offA, offB,
               TmaxT, table, row, nf, nh, idxA_sb, idxB_sb, er_tile,
               ident_sb, out_writer):
    """Shared per-tile edge pipeline for both layers.

    row: table row width; nf: feature count (256 / 32); nh: heads (8 / 1).
    er_tile: [P, nh] AP holding er[dst] for this tile.
    out_writer(acc): consume the [P, nf+nh] PSUM accumulation.
    """
    TtA, TtB = int(TA[t]), int(TB[t])
    TT = TtA + TtB
    g = pool.tile([P, TmaxT * row], F32, tag="g")
    gs = g[:]
    if TtA:
        nc.gpsimd.dma_gather(
            out_ap=_ap(gs, 0, [[row, TtA], [1, row]]),
            in_ap=table[0:SPLIT_ROW + 1, :],
            idxs_ap=idxA_sb[:, int(offA[t]) * 8:(int(offA[t]) + TtA) * 8],
            num_idxs=TtA * P,
            num_idxs_reg=TtA * P,
            elem_size=row,
            queue_num=(t % 2),
            single_packet=False,
        )
    if TtB:
        nc.gpsimd.dma_gather(
            out_ap=_ap(gs, TtA * row, [[row, TtB], [1, row]]),
            in_ap=table[SPLIT_ROW + 1:, :],
            idxs_ap=idxB_sb[:, int(offB[t]) * 8:(int(offB[t]) + TtB) * 8],
            num_idxs=TtB * P,
            num_idxs_reg=TtB * P,
            elem_size=row,
            queue_num=2 + (t % 2),
            single_packet=False,
        )
    # logits lt = el[src] + er[dst]
    lt = small.tile([P, TmaxT * nh], F32, tag="lt")
    el_ap = _ap(gs, nf, [[row, TT], [1, nh]])
    er_ap = _ap(er_tile, 0, [[0, TT], [1, nh]])
    lt_ap = _ap(lt, 0, [[nh, TT], [1, nh]])
    nc.vector.tensor_tensor(out=lt_ap, in0=el_ap, in1=er_ap,
                            op=mybir.AluOpType.add)
    lt2 = small.tile([P, TmaxT * nh], F32, tag="lt2")
    nc.vector.tensor_scalar_mul(lt2[:, :TT * nh], lt[:, :TT * nh], NEG_SLOPE)
    nc.vector.tensor_tensor(out=lt[:, :TT * nh], in0=lt[:, :TT * nh],
                            in1=lt2[:, :TT * nh], op=mybir.AluOpType.max)
    al_ap = _ap(gs, nf, [[row, TT], [1, nh]])
    nc.scalar.activation(out=al_ap, in_=lt_ap,
                         func=mybir.ActivationFunctionType.Exp)
    f_ap = _ap(gs, 0, [[row, TT], [D1, nh], [1, D1]] if nh > 1
               else [[row, TT], [1, nf]])
    ab_ap = _ap(gs, nf, [[row, TT], [1, nh], [0, D1]] if nh > 1
                else [[row, TT], [0, nf]])
    nc.vector.tensor_tensor(out=f_ap, in0=f_ap, in1=ab_ap,
                            op=mybir.AluOpType.mult)
    acc = psum.tile([P, nf + nh], F32, tag="acc")
    gv = gs.rearrange("p (c f) -> p c f", f=row)
    for c in range(TT):
        nc.tensor.matmul(acc[:], ident_sb, gv[:, c, 0:nf + nh],
                         start=(c == 0), stop=(c == TT - 1))
    out_writer(acc)


def _build_launch1(TA, TB):
    TSA, TSB = int(TA.sum()), int(TB.sum())
    offA = np.concatenate([[0], np.cumsum(TA)])
    offB = np.concatenate([[0], np.cumsum(TB)])
    TmaxT = int((TA + TB).max())
    nc = bacc.Bacc("TRN2", target_bir_lowering=False, debug=False,
                   num_devices=NCORES, num_swdge_queues=4)
    xt = nc.dram_tensor("xt", [GBLOCKS, P, P], F32, kind="ExternalInput")
    w1aug = nc.dram_tensor("w1aug", [P, 272], F32, kind="ExternalInput")
    w2aug = nc.dram_tensor("w2aug", [P, 2 * 34], F32, kind="ExternalInput")
    identin = nc.dram_tensor("identin", [P, P], F32, kind="ExternalInput")
    sentin = nc.dram_tensor("sentin", [1, ROW1], F32, kind="ExternalInput")
    idxain = nc.dram_tensor("idxain", [P, 8 * TSA], I16, kind="ExternalInput")
    idxbin = nc.dram_tensor("idxbin", [P, 8 * TSB], I16, kind="ExternalInput")
    ertin = nc.dram_tensor("ertin", [P, TILES], I32, kind="ExternalInput")
    f2out = nc.dram_tensor("f2out", [NPC, 34], F32, kind="ExternalOutput")
    table = nc.dram_tensor("table", [NTAB, ROW1], F32, kind="Internal")

    idxa_sb = nc.alloc_sbuf_tensor("idxa_sb", [P, 8 * TSA], I16).ap()
    idxb_sb = nc.alloc_sbuf_tensor("idxb_sb", [P, 8 * TSB], I16).ap()
    ert_sb = nc.alloc_sbuf_tensor("ert_sb", [P, TILES], I32).ap()
    ident_sb = nc.alloc_sbuf_tensor("ident_sb", [P, P], F32).ap()
    w2_sb = nc.alloc_sbuf_tensor("w2_sb", [P, 2 * 34], F32).ap()

    # ---- phase 1: projection builds the node table --------------------------
    with tile.TileContext(nc) as tc:
        with (
            tc.tile_pool(name="p1sbuf", bufs=3) as pool,
            tc.tile_pool(name="p1psum", bufs=4, space="PSUM") as psum,
            tc.tile_pool(name="p1const", bufs=1) as consts,
        ):
            w1_sb = consts.tile([P, 272], F32)
            nc.sync.dma_start(out=w1_sb[:], in_=w1aug[:])
            nc.sync.dma_start(out=ident_sb, in_=identin[:])
            nc.sync.dma_start(out=w2_sb, in_=w2aug[:])
            nc.sync.dma_start(out=idxa_sb, in_=idxain[:])
            nc.sync.dma_start(out=idxb_sb, in_=idxbin[:])
            nc.sync.dma_start(out=ert_sb, in_=ertin[:])
            sent_sb = consts.tile([1, ROW1], F32)
            nc.sync.dma_start(out=sent_sb[:], in_=sentin[:])
            nc.sync.dma_start(out=table[SENT_A:SENT_A + 1, :], in_=sent_sb[:])
            nc.sync.dma_start(out=table[SENT_B:SENT_B + 1, :], in_=sent_sb[:])
            for b in range(GBLOCKS):
                xtile = pool.tile([P, P], F32, tag="xt")
                nc.sync.dma_start(out=xtile[:], in_=xt[b])
                pp = psum.tile([P, 272], F32, tag="pp")
                nc.tensor.matmul(pp[:], xtile[:], w1_sb[:],
                                 start=True, stop=True)
                fo = pool.tile([P, 272], F32, tag="fo")
                nc.scalar.activation(out=fo[:], in_=pp[:],
                                     func=mybir.ActivationFunctionType.Copy)
                r0 = int(_new_row(b * P))
                nc.sync.dma_start(out=table[r0:r0 + P, 0:272], in_=fo[:])

    # ---- phase 2: layer-1 edges + layer-2 projection ------------------------
    with tile.TileContext(nc) as tc:
        with (
            tc.tile_pool(name="p2sbuf", bufs=2) as pool,
            tc.tile_pool(name="p2small", bufs=3) as small,
            tc.tile_pool(name="p2psum", bufs=2, space="PSUM") as psum,
            tc.tile_pool(name="p2psumT", bufs=2, space="PSUM") as psumT,
            tc.tile_pool(name="p2psum2", bufs=2, space="PSUM") as psum2,
        ):
            for t in range(TILES):
                er_tile = small.tile([P, H1], F32, tag="er")
                nc.gpsimd.indirect_dma_start(
                    out=er_tile[:], out_offset=None, in_=table[:],
                    in_offset=bass.IndirectOffsetOnAxis(
                        ap=ert_sb[:, t:t + 1], axis=0),
                    element_offset=264,
                )

                def writer(acc, t=t):
                    rec = small.tile([P, H1], F32, tag="rec")
                    nc.vector.reciprocal(rec[:], acc[:, 256:264])
                    h1 = pool.tile([P, 256], F32, tag="h1")
                    acc_f = _ap(acc, 0, [[D1, H1], [1, D1]])
                    rb_ap = _ap(rec, 0, [[1, H1], [0, D1]])
                    h1_ap = _ap(h1, 0, [[D1, H1], [1, D1]])
                    nc.vector.tensor_tensor(out=h1_ap, in0=acc_f, in1=rb_ap,
                                            op=mybir.AluOpType.mult)
                    e1 = pool.tile([P, 256], F32, tag="e1")
                    nc.vector.tensor_scalar_min(e1[:], h1[:], 0.0)
                    nc.scalar.activation(out=e1[:], in_=e1[:],
                                         func=mybir.ActivationFunctionType.Exp)
                    nc.vector.tensor_scalar_add(e1[:], e1[:], -1.0)
                    nc.vector.tensor_tensor(out=h1[:], in0=h1[:], in1=e1[:],
                                            op=mybir.AluOpType.max)
                    f2p = psum2.tile([P, 34], F32, tag="f2p")
                    for k in range(2):
                        tp = psumT.tile([P, P], F32, tag="tp")
                        nc.tensor.transpose(out=tp[:],
                                            in_=h1[:, k * P:(k + 1) * P],
                                            identity=ident_sb)
                        h1t = small.tile([P, P], F32, tag="h1t")
                        nc.vector.tensor_copy(out=h1t[:], in_=tp[:])
                        nc.tensor.matmul(f2p[:], h1t[:],
                                         w2_sb[:, k * 34:(k + 1) * 34],
                                         start=(k == 0), stop=(k == 1))
                    f2s = small.tile([P, 34], F32, tag="f2s")
                    nc.scalar.activation(out=f2s[:], in_=f2p[:],
                                         func=mybir.ActivationFunctionType.Copy)
                    nc.sync.dma_start(out=f2out[t * P:(t + 1) * P, :],
                                      in_=f2s[:])

                _edge_tile(nc, pool, small, psum, tc, t=t, TA=TA, TB=TB,
                           offA=offA, offB=offB, TmaxT=TmaxT, table=table,
                           row=ROW1, nf=256, nh=H1, idxA_sb=idxa_sb,
                           idxB_sb=idxb_sb, er_tile=er_tile[:],
                           ident_sb=ident_sb, out_writer=writer)
    nc.compile()
    return nc


def _build_launch2(TA, TB):
    TSA, TSB = int(TA.sum()), int(TB.sum())
    offA = np.concatenate([[0], np.cumsum(TA)])
    offB = np.concatenate([[0], np.cumsum(TB)])
    TmaxT = int((TA + TB).max())
    nc = bacc.Bacc("TRN2", target_bir_lowering=False, debug=False,
                   num_devices=NCORES, num_swdge_queues=4)
    table2 = nc.dram_tensor("table2", [NTAB, ROW2], F32, kind="ExternalInput")
    idxain = nc.dram_tensor("idxain", [P, 8 * TSA], I16, kind="ExternalInput")
    idxbin = nc.dram_tensor("idxbin", [P, 8 * TSB], I16, kind="ExternalInput")
    er2in = nc.dram_tensor("er2in", [P, TILES], F32, kind="ExternalInput")
    identin = nc.dram_tensor("identin", [P, P], F32, kind="ExternalInput")
    outbuf = nc.dram_tensor("outbuf", [NPC, 32], F32, kind="ExternalOutput")

    with tile.TileContext(nc) as tc:
        with (
            tc.tile_pool(name="l2sbuf", bufs=2) as pool,
            tc.tile_pool(name="l2small", bufs=3) as small,
            tc.tile_pool(name="l2psum", bufs=3, space="PSUM") as psum,
            tc.tile_pool(name="l2const", bufs=1) as consts,
        ):
            ident_sb = consts.tile([P, P], F32)
            nc.sync.dma_start(out=ident_sb[:], in_=identin[:])
            idxa_sb = consts.tile([P, 8 * TSA], I16)
            nc.sync.dma_start(out=idxa_sb[:], in_=idxain[:])
            idxb_sb = consts.tile([P, 8 * TSB], I16)
            nc.sync.dma_start(out=idxb_sb[:], in_=idxbin[:])
            er2_sb = consts.tile([P, TILES], F32)
            nc.sync.dma_start(out=er2_sb[:], in_=er2in[:])
            for t in range(TILES):
                def writer(acc, t=t):
                    rec = small.tile([P, 1], F32, tag="rec")
                    nc.vector.reciprocal(rec[:], acc[:, 32:33])
                    o2 = small.tile([P, 32], F32, tag="o2")
                    nc.vector.tensor_scalar_mul(o2[:], acc[:, 0:32],
                                                rec[:, 0:1])
                    nc.sync.dma_start(out=outbuf[t * P:(t + 1) * P, :],
                                      in_=o2[:])

                _edge_tile(nc, pool, small, psum, tc, t=t, TA=TA, TB=TB,
                           offA=offA, offB=offB, TmaxT=TmaxT, table=table2,
                           row=ROW2, nf=32, nh=1, idxA_sb=idxa_sb[:],
                           idxB_sb=idxb_sb[:],
                           er_tile=er2_sb[:, t:t + 1],
                           ident_sb=ident_sb[:], out_writer=writer)
    nc.compile()
    return nc


# ----------------------------------------------------------------------------
# entry point
# ----------------------------------------------------------------------------

_CACHE = {}
PROFILE = False
LAST_EXEC_NS = []


def _run(nc, in_maps, tag):
    if PROFILE:
        import tempfile
        res = run_bass_kernel_spmd(
            nc, in_maps, core_ids=list(range(NCORES)), trace=True,
            tmpdir=tempfile.mkdtemp(prefix=f"gat_{tag}_"),
        )
        LAST_EXEC_NS.append((tag, res.exec_time_ns))
        return res
    return run_bass_kernel_spmd(nc, in_maps, core_ids=list(range(NCORES)))


def kernel(inputs, src, dst, W1, al1, ar1, b1, W2, al2, ar2, b2):
    inputs = np.asarray(inputs, np.float32)
    src = np.asarray(src).astype(np.int64)
    dst = np.asarray(dst).astype(np.int64)
    W1 = np.asarray(W1, np.float32)
    W2 = np.asarray(W2, np.float32)
    al1 = np.asarray(al1, np.float32)
    ar1 = np.asarray(ar1, np.float32)
    al2 = np.asarray(al2, np.float32)
    ar2 = np.asarray(ar2, np.float32)

    prep = _prep(src, dst)
    TA, TB = prep["TA"], prep["TB"]
    newid = prep["newid"]

    key = (tuple(TA.tolist()), tuple(TB.tolist()))
    if key not in _CACHE:
        _CACHE[key] = (_build_launch1(TA, TB), _build_launch2(TA, TB))
    nc1, nc2 = _CACHE[key]

    wl1 = np.einsum("khd,hd->kh", W1.reshape(128, H1, D1), al1)
    wr1 = np.einsum("khd,hd->kh", W1.reshape(128, H1, D1), ar1)
    w1aug = np.concatenate([W1, wl1, wr1], axis=1).astype(np.float32)
    wl2 = np.einsum("khd,hd->kh", W2.reshape(256, 1, 32), al2)
    wr2 = np.einsum("khd,hd->kh", W2.reshape(256, 1, 32), ar2)
    w2a = np.concatenate([W2, wl2, wr2], axis=1).astype(np.float32)
    w2aug = np.concatenate([w2a[:P], w2a[P:]], axis=1)

    x_perm = np.zeros((NPAD, 128), np.float32)
    x_perm[newid] = inputs
    xt_all = np.ascontiguousarray(
        x_perm.reshape(GBLOCKS, P, 128).transpose(0, 2, 1)
    )
    identity = np.eye(P, dtype=np.float32)
    sent = np.zeros((1, ROW1), np.float32)
    sent[0, 256:264] = SENT_EL

    in_maps1 = []
    for c in range(NCORES):
        in_maps1.append({
            "xt": xt_all, "w1aug": w1aug, "w2aug": w2aug,
            "identin": identity, "sentin": sent,
            "idxain": np.ascontiguousarray(prep["idxA"][c]),
            "idxbin": np.ascontiguousarray(prep["idxB"][c]),
            "ertin": prep["ert"][c],
        })
    res1 = _run(nc1, in_maps1, "l1")

    f2_by_newid = np.concatenate(
        [res1.results[c]["f2out"] for c in range(NCORES)], axis=0
    )  # [NPAD, 34]
    tab2 = np.zeros((NTAB, ROW2), np.float32)
    tab2[_new_row(np.arange(NPAD)), 0:34] = f2_by_newid
    tab2[SENT_A, 32] = SENT_EL
    tab2[SENT_B, 32] = SENT_EL
    in_maps2 = []
    for c in range(NCORES):
        own = tab2[_new_row(np.arange(c * NPC, (c + 1) * NPC)), 33]
        er2 = np.ascontiguousarray(own.reshape(TILES, P).T.astype(np.float32))
        in_maps2.append({
            "table2": tab2,
            "idxain": np.ascontiguousarray(prep["idxA"][c]),
            "idxbin": np.ascontiguousarray(prep["idxB"][c]),
            "er2in": er2,
            "identin": identity,
        })
    res2 = _run(nc2, in_maps2, "l2")

    out_by_newid = np.concatenate(
        [res2.results[c]["outbuf"] for c in range(NCORES)], axis=0
    )
    return np.ascontiguousarray(out_by_newid[newid]).astype(np.float32)


# revision 8
# speedup vs baseline: 1.2818x; 1.2818x over previous
"""2-layer GAT on 8 Trainium2 NeuronCores (Bass/Tile).

Sharding: nodes sorted by (in-degree, low-half-src count), snake-dealt
across 8 cores (6250 -> padded 6272/core), tiled 128/tile (49 tiles);
partition j of tile t owns one dst node, its in-edges occupy slots
(chunk c, partition j). Node table in HBM (row = [f 256 | el 8 | er 8 |
pad] f32, 1280B) is built by the projection matmul x @ [W1|W1.al1|W1.ar1].
Edge rows are fetched with the 16-lane dma_gather custom DMA; its int16
index limit (<32768 rows) is beaten by splitting each tile's slots into
two grids gathered from two table views (rows 0..25088 and 25089..50177),
with per-node slot counts equalized by the (deg, a)-sort. alpha =
exp(leaky_relu(el[src]+er[dst])) (no max-subtraction: logits are small);
alpha overwrites the el column so one identity-matmul per chunk
accumulates [sum(alpha*f)|sum(alpha)] in PSUM; divide, ELU. Layer-2
projection h1 @ [W2|wl2|wr2] per tile; the host assembles the full
256B-row layer-2 table for launch 2 (same grids). Padding slots point at
sentinel rows (f=0, el=-300 -> alpha ~= 0).
"""
import sys

sys.path.insert(0, "/opt/trn_rl_repo")

import numpy as np

import concourse.bass as bass
import concourse.bacc as bacc
import concourse.tile as tile
from concourse import mybir
from concourse.bass_utils import run_bass_kernel_spmd

N = 50000
E = 800000
P = 128
NCORES = 8
TILES = 49
NPC = TILES * P                  # 6272
NPAD = NCORES * NPC              # 50176
GBLOCKS = NPAD // P              # 392
SPLIT_ROW = 25088                # sentinel-A row; B view starts at 25089
NTAB = NPAD + 2                  # 50178
SENT_A = SPLIT_ROW
SENT_B = NTAB - 1
ROW1 = 320                       # [f 256 | el 8 | er 8 | pad 48]  (1280B)
ROW2 = 64                        # [f2 32 | el2 1 | er2 1 | pad]   (256B)
H1, D1 = 8, 32
NEG_SLOPE = 0.2
SENT_EL = -300.0
F32 = mybir.dt.float32
I16 = mybir.dt.int16
I32 = mybir.dt.int32


def _new_row(r):
    return r + (r >= SPLIT_ROW)


def _ap(t, off, dims):
    s = t[:] if not isinstance(t, bass.AP) else t
    return bass.AP(tensor=s.tensor, offset=s.offset + off, ap=[s.ap[0]] + dims)


# ----------------------------------------------------------------------------
# host preprocessing
# ----------------------------------------------------------------------------

def _prep(src, dst):
    deg = np.bincount(dst, minlength=N)
    # a = number of in-edges whose src lands in the low table half.
    # Low half = new ids < SPLIT_ROW; new ids depend on this sort, so
    # bootstrap: sort by degree first to fix newid, then a is known only
    # after... break the cycle by sorting on (deg, a_orig) where a_orig is
    # computed from the degree-only assignment.
    order0 = np.argsort(-deg, kind="stable")
    pat = np.concatenate([np.arange(NCORES), np.arange(NCORES - 1, -1, -1)])
    core_of_pos = pat[np.arange(N) % (2 * NCORES)]
    newid0 = np.empty(N, np.int64)
    for c in range(NCORES):
        nodes_c = order0[core_of_pos == c]
        newid0[nodes_c] = c * NPC + np.arange(len(nodes_c))
    low0 = (newid0[src] < SPLIT_ROW).astype(np.int64)
    a_of = np.bincount(dst, weights=low0, minlength=N).astype(np.int64)

    # final order: degree desc, then a desc (keeps per-tile (deg, a) tight)
    order = np.lexsort((-a_of, -deg))
    newid = np.empty(N, np.int64)
    for c in range(NCORES):
        nodes_c = order[core_of_pos == c]
        newid[nodes_c] = c * NPC + np.arange(len(nodes_c))

    nd = newid[dst]
    ns = newid[src]
    low = ns < SPLIT_ROW

    # per-dst counts of low/high srcs
    aA = np.bincount(nd, weights=low, minlength=NPAD).astype(np.int64)
    aB = np.bincount(nd, weights=~low, minlength=NPAD).astype(np.int64)
    TA = aA.reshape(NCORES, TILES, P).max(axis=(0, 2)).astype(np.int64)
    TB = aB.reshape(NCORES, TILES, P).max(axis=(0, 2)).astype(np.int64)
    TB = np.maximum(TB, 1)       # ensure >=1 chunk per tile overall

    # slot assignment: sort edges by (nd, high?) so each dst's A-edges come
    # first; slot k within the group.
    keys = nd * 2 + (~low)
    o = np.argsort(keys, kind="stable")
    nd_s, ns_s, low_s = nd[o], ns[o], low[o]
    ks = np.zeros(E, np.int64)
    kk = np.argsort(o, kind="stable")  # not needed; compute group ranks:
    first = np.searchsorted(keys[o], np.arange(2 * NPAD), side="left")
    ks = np.arange(E) - first[keys[o]]

    offA = np.concatenate([[0], np.cumsum(TA)])
    offB = np.concatenate([[0], np.cumsum(TB)])
    TSA, TSB = int(offA[-1]), int(offB[-1])

    c_s = nd_s // NPC
    t_s = (nd_s % NPC) // P
    j_s = nd_s % P

    # packed int16 index streams, [NCORES, 16, 8*TS]
    idxA = np.full((NCORES, 16, 8 * TSA), SENT_A, np.int16)
    idxB = np.full((NCORES, 16, 8 * TSB), SENT_B - (SPLIT_ROW + 1), np.int16)
    # flat slot id within the per-tile grid: i = c*128 + j  (c = chunk)
    iA = (offA[t_s] + ks) * P + j_s
    iB = (offB[t_s] + ks) * P + j_s
    vA = ns_s                        # A view row == new id (< SPLIT_ROW)
    vB = ns_s + 1 - (SPLIT_ROW + 1)  # B view: row = ns+1, rebased
    m = low_s.astype(bool)
    idxA[c_s[m], iA[m] % 16, iA[m] // 16] = vA[m].astype(np.int16)
    idxB[c_s[~m], iB[~m] % 16, iB[~m] // 16] = vB[~m].astype(np.int16)
    idxA = np.tile(idxA, (1, 8, 1))
    idxB = np.tile(idxB, (1, 8, 1))

    # per-core table row of own node (t, j) for the er fetch
    own = np.arange(NPAD).reshape(NCORES, TILES, P)
    ert = _new_row(own).transpose(0, 2, 1).astype(np.int32)  # [NC, P, TILES]

    return {"newid": newid, "TA": TA, "TB": TB, "idxA": idxA, "idxB": idxB,
            "ert": np.ascontiguousarray(ert)}


# ----------------------------------------------------------------------------
# launch builders
# ----------------------------------------------------------------------------

_QCTR = [0]


def _edge_tile(nc, pool, small, psum, tc, *, t, TA, TB, offA, offB,
               TmaxT, table, row, nf, nh, idxA_sb, idxB_sb, er_tile,
               ident_sb, out_writer):
    """Shared per-tile edge pipeline for both layers.

    row: table row width; nf: feature count (256 / 32); nh: heads (8 / 1).
    er_tile: [P, nh] AP holding er[dst] for this tile.
    out_writer(acc): consume the [P, nf+nh] PSUM accumulation.
    """
    TtA, TtB = int(TA[t]), int(TB[t])
    TT = TtA + TtB
    g = pool.tile([P, TmaxT * row], F32, tag="g")
    gs = g[:]
    # dma_gather with single_packet=True crashes above ~1024 idxs; split
    # each grid into <=8-chunk (1024-row) sub-gathers, round-robin queues.
    MAXC = 8
    for base_t, view_lo, off, idx_sb, TtX in (
        (0, True, offA, idxA_sb, TtA),
        (TtA, False, offB, idxB_sb, TtB),
    ):
        for s0 in range(0, TtX, MAXC):
            sn = min(MAXC, TtX - s0)
            col0 = (int(off[t]) + s0) * 8
            nc.gpsimd.dma_gather(
                out_ap=_ap(gs, (base_t + s0) * row, [[row, sn], [1, row]]),
                in_ap=(table[0:SPLIT_ROW + 1, :] if view_lo
                       else table[SPLIT_ROW + 1:, :]),
                idxs_ap=idx_sb[:, col0:col0 + sn * 8],
                num_idxs=sn * P,
                num_idxs_reg=sn * P,
                elem_size=row,
                queue_num=_QCTR[0] % 4,
                single_packet=True,
            )
            _QCTR[0] += 1
    # logits lt = el[src] + er[dst]
    lt = small.tile([P, TmaxT * nh], F32, tag="lt")
    el_ap = _ap(gs, nf, [[row, TT], [1, nh]])
    er_ap = _ap(er_tile, 0, [[0, TT], [1, nh]])
    lt_ap = _ap(lt, 0, [[nh, TT], [1, nh]])
    nc.vector.tensor_tensor(out=lt_ap, in0=el_ap, in1=er_ap,
                            op=mybir.AluOpType.add)
    lt2 = small.tile([P, TmaxT * nh], F32, tag="lt2")
    nc.vector.tensor_scalar_mul(lt2[:, :TT * nh], lt[:, :TT * nh], NEG_SLOPE)
    nc.vector.tensor_tensor(out=lt[:, :TT * nh], in0=lt[:, :TT * nh],
                            in1=lt2[:, :TT * nh], op=mybir.AluOpType.max)
    al_ap = _ap(gs, nf, [[row, TT], [1, nh]])
    nc.scalar.activation(out=al_ap, in_=lt_ap,
                         func=mybir.ActivationFunctionType.Exp)
    f_ap = _ap(gs, 0, [[row, TT], [D1, nh], [1, D1]] if nh > 1
               else [[row, TT], [1, nf]])
    ab_ap = _ap(gs, nf, [[row, TT], [1, nh], [0, D1]] if nh > 1
                else [[row, TT], [0, nf]])
    nc.vector.tensor_tensor(out=f_ap, in0=f_ap, in1=ab_ap,
                            op=mybir.AluOpType.mult)
    acc = psum.tile([P, nf + nh], F32, tag="acc")
    gv = gs.rearrange("p (c f) -> p c f", f=row)
    for c in range(TT):
        nc.tensor.matmul(acc[:], ident_sb, gv[:, c, 0:nf + nh],
                         start=(c == 0), stop=(c == TT - 1))
    out_writer(acc)


def _build_launch1(TA, TB):
    TSA, TSB = int(TA.sum()), int(TB.sum())
    offA = np.concatenate([[0], np.cumsum(TA)])
    offB = np.concatenate([[0], np.cumsum(TB)])
    TmaxT = int((TA + TB).max())
    nc = bacc.Bacc("TRN2", target_bir_lowering=False, debug=False,
                   num_devices=NCORES, num_swdge_queues=4)
    xt = nc.dram_tensor("xt", [GBLOCKS, P, P], F32, kind="ExternalInput")
    w1aug = nc.dram_tensor("w1aug", [P, 272], F32, kind="ExternalInput")
    w2aug = nc.dram_tensor("w2aug", [P, 2 * 34], F32, kind="ExternalInput")
    identin = nc.dram_tensor("identin", [P, P], F32, kind="ExternalInput")
    sentin = nc.dram_tensor("sentin", [1, ROW1], F32, kind="ExternalInput")
    idxain = nc.dram_tensor("idxain", [P, 8 * TSA], I16, kind="ExternalInput")
    idxbin = nc.dram_tensor("idxbin", [P, 8 * TSB], I16, kind="ExternalInput")
    ertin = nc.dram_tensor("ertin", [P, TILES], I32, kind="ExternalInput")
    f2out = nc.dram_tensor("f2out", [NPC, 34], F32, kind="ExternalOutput")
    table = nc.dram_tensor("table", [NTAB, ROW1], F32, kind="Internal")

    idxa_sb = nc.alloc_sbuf_tensor("idxa_sb", [P, 8 * TSA], I16).ap()
    idxb_sb = nc.alloc_sbuf_tensor("idxb_sb", [P, 8 * TSB], I16).ap()
    ert_sb = nc.alloc_sbuf_tensor("ert_sb", [P, TILES], I32).ap()
    ident_sb = nc.alloc_sbuf_tensor("ident_sb", [P, P], F32).ap()
    w2_sb = nc.alloc_sbuf_tensor("w2_sb", [P, 2 * 34], F32).ap()

    # ---- phase 1: projection builds the node table --------------------------
    with tile.TileContext(nc) as tc:
        with (
            tc.tile_pool(name="p1sbuf", bufs=3) as pool,
            tc.tile_pool(name="p1psum", bufs=4, space="PSUM") as psum,
            tc.tile_pool(name="p1const", bufs=1) as consts,
        ):
            w1_sb = consts.tile([P, 272], F32)
            nc.sync.dma_start(out=w1_sb[:], in_=w1aug[:])
            nc.sync.dma_start(out=ident_sb, in_=identin[:])
            nc.sync.dma_start(out=w2_sb, in_=w2aug[:])
            nc.sync.dma_start(out=idxa_sb, in_=idxain[:])
            nc.sync.dma_start(out=idxb_sb, in_=idxbin[:])
            nc.sync.dma_start(out=ert_sb, in_=ertin[:])
            sent_sb = consts.tile([1, ROW1], F32)
            nc.sync.dma_start(out=sent_sb[:], in_=sentin[:])
            nc.sync.dma_start(out=table[SENT_A:SENT_A + 1, :], in_=sent_sb[:])
            nc.sync.dma_start(out=table[SENT_B:SENT_B + 1, :], in_=sent_sb[:])
            for b in range(GBLOCKS):
                xtile = pool.tile([P, P], F32, tag="xt")
                nc.sync.dma_start(out=xtile[:], in_=xt[b])
                pp = psum.tile([P, 272], F32, tag="pp")
                nc.tensor.matmul(pp[:], xtile[:], w1_sb[:],
                                 start=True, stop=True)
                fo = pool.tile([P, 272], F32, tag="fo")
                nc.scalar.activation(out=fo[:], in_=pp[:],
                                     func=mybir.ActivationFunctionType.Copy)
                r0 = int(_new_row(b * P))
                nc.sync.dma_start(out=table[r0:r0 + P, 0:272], in_=fo[:])

    # ---- phase 2: layer-1 edges + layer-2 projection ------------------------
    with tile.TileContext(nc) as tc:
        with (
            tc.tile_pool(name="p2sbuf", bufs=2) as pool,
            tc.tile_pool(name="p2small", bufs=3) as small,
            tc.tile_pool(name="p2psum", bufs=2, space="PSUM") as psum,
            tc.tile_pool(name="p2psumT", bufs=2, space="PSUM") as psumT,
            tc.tile_pool(name="p2psum2", bufs=2, space="PSUM") as psum2,
        ):
            for t in range(TILES):
                er_tile = small.tile([P, H1], F32, tag="er")
                nc.gpsimd.indirect_dma_start(
                    out=er_tile[:], out_offset=None, in_=table[:],
                    in_offset=bass.IndirectOffsetOnAxis(
                        ap=ert_sb[:, t:t + 1], axis=0),
                    element_offset=264,
                )

                def writer(acc, t=t):
                    rec = small.tile([P, H1], F32, tag="rec")
                    nc.vector.reciprocal(rec[:], acc[:, 256:264])
                    h1 = pool.tile([P, 256], F32, tag="h1")
                    acc_f = _ap(acc, 0, [[D1, H1], [1, D1]])
                    rb_ap = _ap(rec, 0, [[1, H1], [0, D1]])
                    h1_ap = _ap(h1, 0, [[D1, H1], [1, D1]])
                    nc.vector.tensor_tensor(out=h1_ap, in0=acc_f, in1=rb_ap,
                                            op=mybir.AluOpType.mult)
                    e1 = pool.tile([P, 256], F32, tag="e1")
                    nc.vector.tensor_scalar_min(e1[:], h1[:], 0.0)
                    nc.scalar.activation(out=e1[:], in_=e1[:],
                                         func=mybir.ActivationFunctionType.Exp)
                    nc.vector.tensor_scalar_add(e1[:], e1[:], -1.0)
                    nc.vector.tensor_tensor(out=h1[:], in0=h1[:], in1=e1[:],
                                            op=mybir.AluOpType.max)
                    f2p = psum2.tile([P, 34], F32, tag="f2p")
                    for k in range(2):
                        tp = psumT.tile([P, P], F32, tag="tp")
                        nc.tensor.transpose(out=tp[:],
                                            in_=h1[:, k * P:(k + 1) * P],
                                            identity=ident_sb)
                        h1t = small.tile([P, P], F32, tag="h1t")
                        nc.vector.tensor_copy(out=h1t[:], in_=tp[:])
                        nc.tensor.matmul(f2p[:], h1t[:],
                                         w2_sb[:, k * 34:(k + 1) * 34],
                                         start=(k == 0), stop=(k == 1))
                    f2s = small.tile([P, 34], F32, tag="f2s")
                    nc.scalar.activation(out=f2s[:], in_=f2p[:],
                                         func=mybir.ActivationFunctionType.Copy)
                    nc.sync.dma_start(out=f2out[t * P:(t + 1) * P, :],
                                      in_=f2s[:])

                _edge_tile(nc, pool, small, psum, tc, t=t, TA=TA, TB=TB,
                           offA=offA, offB=offB, TmaxT=TmaxT, table=table,
                           row=ROW1, nf=256, nh=H1, idxA_sb=idxa_sb,
                           idxB_sb=idxb_sb, er_tile=er_tile[:],
                           ident_sb=ident_sb, out_writer=writer)
    nc.compile()
    return nc


def _build_launch2(TA, TB):
    TSA, TSB = int(TA.sum()), int(TB.sum())
    offA = np.concatenate([[0], np.cumsum(TA)])
    offB = np.concatenate([[0], np.cumsum(TB)])
    TmaxT = int((TA + TB).max())
    nc = bacc.Bacc("TRN2", target_bir_lowering=False, debug=False,
                   num_devices=NCORES, num_swdge_queues=4)
    table2 = nc.dram_tensor("table2", [NTAB, ROW2], F32, kind="ExternalInput")
    idxain = nc.dram_tensor("idxain", [P, 8 * TSA], I16, kind="ExternalInput")
    idxbin = nc.dram_tensor("idxbin", [P, 8 * TSB], I16, kind="ExternalInput")
    er2in = nc.dram_tensor("er2in", [P, TILES], F32, kind="ExternalInput")
    identin = nc.dram_tensor("identin", [P, P], F32, kind="ExternalInput")
    outbuf = nc.dram_tensor("outbuf", [NPC, 32], F32, kind="ExternalOutput")

    with tile.TileContext(nc) as tc:
        with (
            tc.tile_pool(name="l2sbuf", bufs=2) as pool,
            tc.tile_pool(name="l2small", bufs=3) as small,
            tc.tile_pool(name="l2psum", bufs=3, space="PSUM") as psum,
            tc.tile_pool(name="l2const", bufs=1) as consts,
        ):
            ident_sb = consts.tile([P, P], F32)
            nc.sync.dma_start(out=ident_sb[:], in_=identin[:])
            idxa_sb = consts.tile([P, 8 * TSA], I16)
            nc.sync.dma_start(out=idxa_sb[:], in_=idxain[:])
            idxb_sb = consts.tile([P, 8 * TSB], I16)
            nc.sync.dma_start(out=idxb_sb[:], in_=idxbin[:])
            er2_sb = consts.tile([P, TILES], F32)
            nc.sync.dma_start(out=er2_sb[:], in_=er2in[:])
            for t in range(TILES):
                def writer(acc, t=t):
                    rec = small.tile([P, 1], F32, tag="rec")
                    nc.vector.reciprocal(rec[:], acc[:, 32:33])
                    o2 = small.tile([P, 32], F32, tag="o2")
                    nc.vector.tensor_scalar_mul(o2[:], acc[:, 0:32],
                                                rec[:, 0:1])
                    nc.sync.dma_start(out=outbuf[t * P:(t + 1) * P, :],
                                      in_=o2[:])

                _edge_tile(nc, pool, small, psum, tc, t=t, TA=TA, TB=TB,
                           offA=offA, offB=offB, TmaxT=TmaxT, table=table2,
                           row=ROW2, nf=32, nh=1, idxA_sb=idxa_sb[:],
                           idxB_sb=idxb_sb[:],
                           er_tile=er2_sb[:, t:t + 1],
                           ident_sb=ident_sb[:], out_writer=writer)
    nc.compile()
    return nc


# ----------------------------------------------------------------------------
# entry point
# ----------------------------------------------------------------------------

_CACHE = {}
PROFILE = False
LAST_EXEC_NS = []


def _run(nc, in_maps, tag):
    if PROFILE:
        import tempfile
        res = run_bass_kernel_spmd(
            nc, in_maps, core_ids=list(range(NCORES)), trace=True,
            tmpdir=tempfile.mkdtemp(prefix=f"gat_{tag}_"),
        )
        LAST_EXEC_NS.append((tag, res.exec_time_ns))
        return res
    return run_bass_kernel_spmd(nc, in_maps, core_ids=list(range(NCORES)))


def kernel(inputs, src, dst, W1, al1, ar1, b1, W2, al2, ar2, b2):
    inputs = np.asarray(inputs, np.float32)
    src = np.asarray(src).astype(np.int64)
    dst = np.asarray(dst).astype(np.int64)
    W1 = np.asarray(W1, np.float32)
    W2 = np.asarray(W2, np.float32)
    al1 = np.asarray(al1, np.float32)
    ar1 = np.asarray(ar1, np.float32)
    al2 = np.asarray(al2, np.float32)
    ar2 = np.asarray(ar2, np.float32)

    prep = _prep(src, dst)
    TA, TB = prep["TA"], prep["TB"]
    newid = prep["newid"]

    key = (tuple(TA.tolist()), tuple(TB.tolist()))
    if key not in _CACHE:
        _CACHE[key] = (_build_launch1(TA, TB), _build_launch2(TA, TB))
    nc1, nc2 = _CACHE[key]

    wl1 = np.einsum("khd,hd->kh", W1.reshape(128, H1, D1), al1)
    wr1 = np.einsum("khd,hd->kh", W1.reshape(128, H1, D1), ar1)
    w1aug = np.concatenate([W1, wl1, wr1], axis=1).astype(np.float32)
    wl2 = np.einsum("khd,hd->kh", W2.reshape(256, 1, 32), al2)
    wr2 = np.einsum("khd,hd->kh", W2.reshape(256, 1, 32), ar2)
    w2a = np.concatenate([W2, wl2, wr2], axis=1).astype(np.float32)
    w2aug = np.concatenate([w2a[:P], w2a[P:]], axis=1)

    x_perm = np.zeros((NPAD, 128), np.float32)
    x_perm[newid] = inputs
    xt_all = np.ascontiguousarray(
        x_perm.reshape(GBLOCKS, P, 128).transpose(0, 2, 1)
    )
    identity = np.eye(P, dtype=np.float32)
    sent = np.zeros((1, ROW1), np.float32)
    sent[0, 256:264] = SENT_EL

    in_maps1 = []
    for c in range(NCORES):
        in_maps1.append({
            "xt": xt_all, "w1aug": w1aug, "w2aug": w2aug,
            "identin": identity, "sentin": sent,
            "idxain": np.ascontiguousarray(prep["idxA"][c]),
            "idxbin": np.ascontiguousarray(prep["idxB"][c]),
            "ertin": prep["ert"][c],
        })
    res1 = _run(nc1, in_maps1, "l1")

    f2_by_newid = np.concatenate(
        [res1.results[c]["f2out"] for c in range(NCORES)], axis=0
    )  # [NPAD, 34]
    tab2 = np.zeros((NTAB, ROW2), np.float32)
    tab2[_new_row(np.arange(NPAD)), 0:34] = f2_by_newid
    tab2[SENT_A, 32] = SENT_EL
    tab2[SENT_B, 32] = SENT_EL
    in_maps2 = []
    for c in range(NCORES):
        own = tab2[_new_row(np.arange(c * NPC, (c + 1) * NPC)), 33]
        er2 = np.ascontiguousarray(own.reshape(TILES, P).T.astype(np.float32))
        in_maps2.append({
            "table2": tab2,
            "idxain": np.ascontiguousarray(prep["idxA"][c]),
            "idxbin": np.ascontiguousarray(prep["idxB"][c]),
            "er2in": er2,
            "identin": identity,
        })
    res2 = _run(nc2, in_maps2, "l2")

    out_by_newid = np.concatenate(
        [res2.results[c]["outbuf"] for c in range(NCORES)], axis=0
    )
    return np.ascontiguousarray(out_by_newid[newid]).astype(np.float32)


# revision 10
# speedup vs baseline: 1.2988x; 1.0133x over previous
"""2-layer GAT on 8 Trainium2 NeuronCores (Bass/Tile).

Sharding: nodes sorted by (in-degree, low-half-src count), snake-dealt
across 8 cores (6250 -> padded 6272/core), tiled 128/tile (49 tiles);
partition j of tile t owns one dst node, its in-edges occupy slots
(chunk c, partition j). Node table in HBM (row = [f 256 | el 8 | er 8 |
pad] f32, 1280B) is built by the projection matmul x @ [W1|W1.al1|W1.ar1].
Edge rows are fetched with the 16-lane dma_gather custom DMA; its int16
index limit (<32768 rows) is beaten by splitting each tile's slots into
two grids gathered from two table views (rows 0..25088 and 25089..50177),
with per-node slot counts equalized by the (deg, a)-sort. alpha =
exp(leaky_relu(el[src]+er[dst])) (no max-subtraction: logits are small);
alpha overwrites the el column so one identity-matmul per chunk
accumulates [sum(alpha*f)|sum(alpha)] in PSUM; divide, ELU. Layer-2
projection h1 @ [W2|wl2|wr2] per tile; the host assembles the full
256B-row layer-2 table for launch 2 (same grids). Padding slots point at
sentinel rows (f=0, el=-300 -> alpha ~= 0).
"""
import sys

sys.path.insert(0, "/opt/trn_rl_repo")

import numpy as np

import concourse.bass as bass
import concourse.bacc as bacc
import concourse.tile as tile
from concourse import mybir
from concourse.bass_utils import run_bass_kernel_spmd

N = 50000
E = 800000
P = 128
NCORES = 8
TILES = 49
NPC = TILES * P                  # 6272
NPAD = NCORES * NPC              # 50176
GBLOCKS = NPAD // P              # 392
SPLIT_ROW = 25088                # sentinel-A row; B view starts at 25089
NTAB = NPAD + 2                  # 50178
SENT_A = SPLIT_ROW
SENT_B = NTAB - 1
ROW1 = 384                       # bf16: [f 256 | alpha 8 | pad | el-f32@136 | er-f32@144]  (768B)
EL1_F32 = 136                    # f32-view col of el in a layer-1 row
ER1_F32 = 144                    # f32-view col of er
ER1_BF = 288                     # bf16-elem offset of er region
ROW2 = 64                        # [f2 32 | el2 1 | er2 1 | pad]   (256B)
H1, D1 = 8, 32
NEG_SLOPE = 0.2
SENT_EL = -300.0
F32 = mybir.dt.float32
BF16 = mybir.dt.bfloat16
I16 = mybir.dt.int16
I32 = mybir.dt.int32


def _new_row(r):
    return r + (r >= SPLIT_ROW)


def _ap(t, off, dims):
    s = t[:] if not isinstance(t, bass.AP) else t
    return bass.AP(tensor=s.tensor, offset=s.offset + off, ap=[s.ap[0]] + dims)


# ----------------------------------------------------------------------------
# host preprocessing
# ----------------------------------------------------------------------------

def _prep(src, dst):
    deg = np.bincount(dst, minlength=N)
    # a = number of in-edges whose src lands in the low table half.
    # Low half = new ids < SPLIT_ROW; new ids depend on this sort, so
    # bootstrap: sort by degree first to fix newid, then a is known only
    # after... break the cycle by sorting on (deg, a_orig) where a_orig is
    # computed from the degree-only assignment.
    order0 = np.argsort(-deg, kind="stable")
    pat = np.concatenate([np.arange(NCORES), np.arange(NCORES - 1, -1, -1)])
    core_of_pos = pat[np.arange(N) % (2 * NCORES)]
    newid0 = np.empty(N, np.int64)
    for c in range(NCORES):
        nodes_c = order0[core_of_pos == c]
        newid0[nodes_c] = c * NPC + np.arange(len(nodes_c))
    low0 = (newid0[src] < SPLIT_ROW).astype(np.int64)
    a_of = np.bincount(dst, weights=low0, minlength=N).astype(np.int64)

    # final order: degree desc, then a desc (keeps per-tile (deg, a) tight)
    order = np.lexsort((-a_of, -deg))
    newid = np.empty(N, np.int64)
    for c in range(NCORES):
        nodes_c = order[core_of_pos == c]
        newid[nodes_c] = c * NPC + np.arange(len(nodes_c))

    nd = newid[dst]
    ns = newid[src]
    low = ns < SPLIT_ROW

    # per-dst counts of low/high srcs
    aA = np.bincount(nd, weights=low, minlength=NPAD).astype(np.int64)
    aB = np.bincount(nd, weights=~low, minlength=NPAD).astype(np.int64)
    TA = aA.reshape(NCORES, TILES, P).max(axis=(0, 2)).astype(np.int64)
    TB = aB.reshape(NCORES, TILES, P).max(axis=(0, 2)).astype(np.int64)
    TB = np.maximum(TB, 1)       # ensure >=1 chunk per tile overall

    # slot assignment: sort edges by (nd, high?) so each dst's A-edges come
    # first; slot k within the group.
    keys = nd * 2 + (~low)
    o = np.argsort(keys, kind="stable")
    nd_s, ns_s, low_s = nd[o], ns[o], low[o]
    ks = np.zeros(E, np.int64)
    kk = np.argsort(o, kind="stable")  # not needed; compute group ranks:
    first = np.searchsorted(keys[o], np.arange(2 * NPAD), side="left")
    ks = np.arange(E) - first[keys[o]]

    offA = np.concatenate([[0], np.cumsum(TA)])
    offB = np.concatenate([[0], np.cumsum(TB)])
    TSA, TSB = int(offA[-1]), int(offB[-1])

    c_s = nd_s // NPC
    t_s = (nd_s % NPC) // P
    j_s = nd_s % P

    # packed int16 index streams, [NCORES, 16, 8*TS]
    idxA = np.full((NCORES, 16, 8 * TSA), SENT_A, np.int16)
    idxB = np.full((NCORES, 16, 8 * TSB), SENT_B - (SPLIT_ROW + 1), np.int16)
    # flat slot id within the per-tile grid: i = c*128 + j  (c = chunk)
    iA = (offA[t_s] + ks) * P + j_s
    iB = (offB[t_s] + ks) * P + j_s
    vA = ns_s                        # A view row == new id (< SPLIT_ROW)
    vB = ns_s + 1 - (SPLIT_ROW + 1)  # B view: row = ns+1, rebased
    m = low_s.astype(bool)
    idxA[c_s[m], iA[m] % 16, iA[m] // 16] = vA[m].astype(np.int16)
    idxB[c_s[~m], iB[~m] % 16, iB[~m] // 16] = vB[~m].astype(np.int16)
    idxA = np.tile(idxA, (1, 8, 1))
    idxB = np.tile(idxB, (1, 8, 1))

    # per-core table row of own node (t, j) for the er fetch
    own = np.arange(NPAD).reshape(NCORES, TILES, P)
    ert = _new_row(own).transpose(0, 2, 1).astype(np.int32)  # [NC, P, TILES]

    return {"newid": newid, "TA": TA, "TB": TB, "idxA": idxA, "idxB": idxB,
            "ert": np.ascontiguousarray(ert)}


# ----------------------------------------------------------------------------
# launch builders
# ----------------------------------------------------------------------------

_QCTR = [0]


def _edge_tile(nc, pool, small, psum, tc, *, t, TA, TB, offA, offB,
               TmaxT, table, row, nf, nh, idxA_sb, idxB_sb, er_tile,
               ident_sb, out_writer, bf=False):
    """Shared per-tile edge pipeline for both layers.

    row: table row width; nf: feature count (256 / 32); nh: heads (8 / 1).
    er_tile: [P, nh] AP holding er[dst] for this tile.
    out_writer(acc): consume the [P, nf+nh] PSUM accumulation.
    """
    TtA, TtB = int(TA[t]), int(TB[t])
    TT = TtA + TtB
    gdt = BF16 if bf else F32
    g = pool.tile([P, TmaxT * row], gdt, tag="g")
    gs = g[:]
    # dma_gather with single_packet=True crashes above ~1024 idxs; split
    # each grid into <=8-chunk (1024-row) sub-gathers, round-robin queues.
    MAXC = 8
    for base_t, view_lo, off, idx_sb, TtX in (
        (0, True, offA, idxA_sb, TtA),
        (TtA, False, offB, idxB_sb, TtB),
    ):
        for s0 in range(0, TtX, MAXC):
            sn = min(MAXC, TtX - s0)
            col0 = (int(off[t]) + s0) * 8
            nc.gpsimd.dma_gather(
                out_ap=_ap(gs, (base_t + s0) * row, [[row, sn], [1, row]]),
                in_ap=(table[0:SPLIT_ROW + 1, :] if view_lo
                       else table[SPLIT_ROW + 1:, :]),
                idxs_ap=idx_sb[:, col0:col0 + sn * 8],
                num_idxs=sn * P,
                num_idxs_reg=sn * P,
                elem_size=row,
                queue_num=_QCTR[0] % 4,
                single_packet=True,
            )
            _QCTR[0] += 1
    # logits lt = el[src] + er[dst]
    lt = small.tile([P, TmaxT * nh], F32, tag="lt")
    if bf:
        el_ap = _ap(gs.bitcast(F32), EL1_F32, [[row // 2, TT], [1, nh]])
    else:
        el_ap = _ap(gs, nf, [[row, TT], [1, nh]])
    er_ap = _ap(er_tile, 0, [[0, TT], [1, nh]])
    lt_ap = _ap(lt, 0, [[nh, TT], [1, nh]])
    nc.vector.tensor_tensor(out=lt_ap, in0=el_ap, in1=er_ap,
                            op=mybir.AluOpType.add)
    lt2 = small.tile([P, TmaxT * nh], F32, tag="lt2")
    nc.vector.tensor_scalar_mul(lt2[:, :TT * nh], lt[:, :TT * nh], NEG_SLOPE)
    nc.vector.tensor_tensor(out=lt[:, :TT * nh], in0=lt[:, :TT * nh],
                            in1=lt2[:, :TT * nh], op=mybir.AluOpType.max)
    al_ap = _ap(gs, nf, [[row, TT], [1, nh]])
    nc.scalar.activation(out=al_ap, in_=lt_ap,
                         func=mybir.ActivationFunctionType.Exp)
    f_ap = _ap(gs, 0, [[row, TT], [D1, nh], [1, D1]] if nh > 1
               else [[row, TT], [1, nf]])
    ab_ap = _ap(gs, nf, [[row, TT], [1, nh], [0, D1]] if nh > 1
                else [[row, TT], [0, nf]])
    nc.vector.tensor_tensor(out=f_ap, in0=f_ap, in1=ab_ap,
                            op=mybir.AluOpType.mult)
    acc = psum.tile([P, nf + nh], F32, tag="acc")
    gv = gs.rearrange("p (c f) -> p c f", f=row)
    for c in range(TT):
        nc.tensor.matmul(acc[:], ident_sb, gv[:, c, 0:nf + nh],
                         start=(c == 0), stop=(c == TT - 1))
    out_writer(acc)


def _build_launch1(TA, TB):
    TSA, TSB = int(TA.sum()), int(TB.sum())
    offA = np.concatenate([[0], np.cumsum(TA)])
    offB = np.concatenate([[0], np.cumsum(TB)])
    TmaxT = int((TA + TB).max())
    nc = bacc.Bacc("TRN2", target_bir_lowering=False, debug=False,
                   num_devices=NCORES, num_swdge_queues=4)
    xt = nc.dram_tensor("xt", [GBLOCKS, P, P], F32, kind="ExternalInput")
    w1aug = nc.dram_tensor("w1aug", [P, 272], F32, kind="ExternalInput")
    w2aug = nc.dram_tensor("w2aug", [P, 2 * 34], F32, kind="ExternalInput")
    identin = nc.dram_tensor("identin", [P, P], F32, kind="ExternalInput")
    idxain = nc.dram_tensor("idxain", [P, 8 * TSA], I16, kind="ExternalInput")
    idxbin = nc.dram_tensor("idxbin", [P, 8 * TSB], I16, kind="ExternalInput")
    ertin = nc.dram_tensor("ertin", [P, TILES], I32, kind="ExternalInput")
    f2out = nc.dram_tensor("f2out", [NPC, 34], F32, kind="ExternalOutput")
    table = nc.dram_tensor("table", [NTAB, ROW1], BF16, kind="Internal")

    idxa_sb = nc.alloc_sbuf_tensor("idxa_sb", [P, 8 * TSA], I16).ap()
    idxb_sb = nc.alloc_sbuf_tensor("idxb_sb", [P, 8 * TSB], I16).ap()
    ert_sb = nc.alloc_sbuf_tensor("ert_sb", [P, TILES], I32).ap()
    ident_sb = nc.alloc_sbuf_tensor("ident_sb", [P, P], F32).ap()
    ident16 = nc.alloc_sbuf_tensor("ident16", [P, P], BF16).ap()
    w2_sb = nc.alloc_sbuf_tensor("w2_sb", [P, 2 * 34], F32).ap()

    # ---- phase 1: projection builds the node table --------------------------
    with tile.TileContext(nc) as tc:
        with (
            tc.tile_pool(name="p1sbuf", bufs=3) as pool,
            tc.tile_pool(name="p1psum", bufs=4, space="PSUM") as psum,
            tc.tile_pool(name="p1const", bufs=1) as consts,
        ):
            w1_sb = consts.tile([P, 272], F32)
            nc.sync.dma_start(out=w1_sb[:], in_=w1aug[:])
            nc.sync.dma_start(out=ident_sb, in_=identin[:])
            nc.vector.tensor_copy(out=ident16, in_=ident_sb)
            nc.sync.dma_start(out=w2_sb, in_=w2aug[:])
            nc.sync.dma_start(out=idxa_sb, in_=idxain[:])
            nc.sync.dma_start(out=idxb_sb, in_=idxbin[:])
            nc.sync.dma_start(out=ert_sb, in_=ertin[:])
            sent_sb = consts.tile([1, ROW1], BF16)
            nc.vector.memset(sent_sb[:], 0.0)
            nc.vector.memset(
                sent_sb[:].bitcast(F32)[:, EL1_F32:EL1_F32 + H1], SENT_EL)
            nc.sync.dma_start(out=table[SENT_A:SENT_A + 1, :], in_=sent_sb[:])
            nc.sync.dma_start(out=table[SENT_B:SENT_B + 1, :], in_=sent_sb[:])
            for b in range(GBLOCKS):
                xtile = pool.tile([P, P], F32, tag="xt")
                nc.sync.dma_start(out=xtile[:], in_=xt[b])
                pp = psum.tile([P, 272], F32, tag="pp")
                nc.tensor.matmul(pp[:], xtile[:], w1_sb[:],
                                 start=True, stop=True)
                fo = pool.tile([P, ROW1], BF16, tag="fo")
                nc.scalar.activation(out=fo[:, 0:256], in_=pp[:, 0:256],
                                     func=mybir.ActivationFunctionType.Copy)
                nc.vector.tensor_copy(
                    out=fo[:].bitcast(F32)[:, EL1_F32:EL1_F32 + 16],
                    in_=pp[:, 256:272])
                r0 = int(_new_row(b * P))
                nc.sync.dma_start(out=table[r0:r0 + P, :], in_=fo[:])

    # ---- phase 2: layer-1 edges + layer-2 projection ------------------------
    with tile.TileContext(nc) as tc:
        with (
            tc.tile_pool(name="p2sbuf", bufs=2) as pool,
            tc.tile_pool(name="p2small", bufs=3) as small,
            tc.tile_pool(name="p2psum", bufs=2, space="PSUM") as psum,
            tc.tile_pool(name="p2psumT", bufs=2, space="PSUM") as psumT,
            tc.tile_pool(name="p2psum2", bufs=2, space="PSUM") as psum2,
        ):
            for t in range(TILES):
                er16 = small.tile([P, 2 * H1], BF16, tag="er")
                nc.gpsimd.indirect_dma_start(
                    out=er16[:], out_offset=None, in_=table[:],
                    in_offset=bass.IndirectOffsetOnAxis(
                        ap=ert_sb[:, t:t + 1], axis=0),
                    element_offset=ER1_BF,
                )
                er_tile = er16

                def writer(acc, t=t):
                    rec = small.tile([P, H1], F32, tag="rec")
                    nc.vector.reciprocal(rec[:], acc[:, 256:264])
                    h1 = pool.tile([P, 256], F32, tag="h1")
                    acc_f = _ap(acc, 0, [[D1, H1], [1, D1]])
                    rb_ap = _ap(rec, 0, [[1, H1], [0, D1]])
                    h1_ap = _ap(h1, 0, [[D1, H1], [1, D1]])
                    nc.vector.tensor_tensor(out=h1_ap, in0=acc_f, in1=rb_ap,
                                            op=mybir.AluOpType.mult)
                    e1 = pool.tile([P, 256], F32, tag="e1")
                    nc.vector.tensor_scalar_min(e1[:], h1[:], 0.0)
                    nc.scalar.activation(out=e1[:], in_=e1[:],
                                         func=mybir.ActivationFunctionType.Exp)
                    nc.vector.tensor_scalar_add(e1[:], e1[:], -1.0)
                    nc.vector.tensor_tensor(out=h1[:], in0=h1[:], in1=e1[:],
                                            op=mybir.AluOpType.max)
                    f2p = psum2.tile([P, 34], F32, tag="f2p")
                    for k in range(2):
                        tp = psumT.tile([P, P], F32, tag="tp")
                        nc.tensor.transpose(out=tp[:],
                                            in_=h1[:, k * P:(k + 1) * P],
                                            identity=ident_sb)
                        h1t = small.tile([P, P], F32, tag="h1t")
                        nc.vector.tensor_copy(out=h1t[:], in_=tp[:])
                        nc.tensor.matmul(f2p[:], h1t[:],
                                         w2_sb[:, k * 34:(k + 1) * 34],
                                         start=(k == 0), stop=(k == 1))
                    f2s = small.tile([P, 34], F32, tag="f2s")
                    nc.scalar.activation(out=f2s[:], in_=f2p[:],
                                         func=mybir.ActivationFunctionType.Copy)
                    nc.sync.dma_start(out=f2out[t * P:(t + 1) * P, :],
                                      in_=f2s[:])

                _edge_tile(nc, pool, small, psum, tc, t=t, TA=TA, TB=TB,
                           offA=offA, offB=offB, TmaxT=TmaxT, table=table,
                           row=ROW1, nf=256, nh=H1, idxA_sb=idxa_sb,
                           idxB_sb=idxb_sb,
                           er_tile=er_tile[:].bitcast(F32),
                           ident_sb=ident16, out_writer=writer, bf=True)
    nc.compile()
    return nc


def _build_launch2(TA, TB):
    TSA, TSB = int(TA.sum()), int(TB.sum())
    offA = np.concatenate([[0], np.cumsum(TA)])
    offB = np.concatenate([[0], np.cumsum(TB)])
    TmaxT = int((TA + TB).max())
    nc = bacc.Bacc("TRN2", target_bir_lowering=False, debug=False,
                   num_devices=NCORES, num_swdge_queues=4)
    table2 = nc.dram_tensor("table2", [NTAB, ROW2], F32, kind="ExternalInput")
    idxain = nc.dram_tensor("idxain", [P, 8 * TSA], I16, kind="ExternalInput")
    idxbin = nc.dram_tensor("idxbin", [P, 8 * TSB], I16, kind="ExternalInput")
    er2in = nc.dram_tensor("er2in", [P, TILES], F32, kind="ExternalInput")
    identin = nc.dram_tensor("identin", [P, P], F32, kind="ExternalInput")
    outbuf = nc.dram_tensor("outbuf", [NPC, 32], F32, kind="ExternalOutput")

    with tile.TileContext(nc) as tc:
        with (
            tc.tile_pool(name="l2sbuf", bufs=2) as pool,
            tc.tile_pool(name="l2small", bufs=3) as small,
            tc.tile_pool(name="l2psum", bufs=3, space="PSUM") as psum,
            tc.tile_pool(name="l2const", bufs=1) as consts,
        ):
            ident_sb = consts.tile([P, P], F32)
            nc.sync.dma_start(out=ident_sb[:], in_=identin[:])
            idxa_sb = consts.tile([P, 8 * TSA], I16)
            nc.sync.dma_start(out=idxa_sb[:], in_=idxain[:])
            idxb_sb = consts.tile([P, 8 * TSB], I16)
            nc.sync.dma_start(out=idxb_sb[:], in_=idxbin[:])
            er2_sb = consts.tile([P, TILES], F32)
            nc.sync.dma_start(out=er2_sb[:], in_=er2in[:])
            for t in range(TILES):
                def writer(acc, t=t):
                    rec = small.tile([P, 1], F32, tag="rec")
                    nc.vector.reciprocal(rec[:], acc[:, 32:33])
                    o2 = small.tile([P, 32], F32, tag="o2")
                    nc.vector.tensor_scalar_mul(o2[:], acc[:, 0:32],
                                                rec[:, 0:1])
                    nc.sync.dma_start(out=outbuf[t * P:(t + 1) * P, :],
                                      in_=o2[:])

                _edge_tile(nc, pool, small, psum, tc, t=t, TA=TA, TB=TB,
                           offA=offA, offB=offB, TmaxT=TmaxT, table=table2,
                           row=ROW2, nf=32, nh=1, idxA_sb=idxa_sb[:],
                           idxB_sb=idxb_sb[:],
                           er_tile=er2_sb[:, t:t + 1],
                           ident_sb=ident_sb[:], out_writer=writer)
    nc.compile()
    return nc


# ----------------------------------------------------------------------------
# entry point
# ----------------------------------------------------------------------------

_CACHE = {}
PROFILE = False
LAST_EXEC_NS = []


def _run(nc, in_maps, tag):
    if PROFILE:
        import tempfile
        res = run_bass_kernel_spmd(
            nc, in_maps, core_ids=list(range(NCORES)), trace=True,
            tmpdir=tempfile.mkdtemp(prefix=f"gat_{tag}_"),
        )
        LAST_EXEC_NS.append((tag, res.exec_time_ns))
        return res
    return run_bass_kernel_spmd(nc, in_maps, core_ids=list(range(NCORES)))


def kernel(inputs, src, dst, W1, al1, ar1, b1, W2, al2, ar2, b2):
    inputs = np.asarray(inputs, np.float32)
    src = np.asarray(src).astype(np.int64)
    dst = np.asarray(dst).astype(np.int64)
    W1 = np.asarray(W1, np.float32)
    W2 = np.asarray(W2, np.float32)
    al1 = np.asarray(al1, np.float32)
    ar1 = np.asarray(ar1, np.float32)
    al2 = np.asarray(al2, np.float32)
    ar2 = np.asarray(ar2, np.float32)

    prep = _prep(src, dst)
    TA, TB = prep["TA"], prep["TB"]
    newid = prep["newid"]

    key = (tuple(TA.tolist()), tuple(TB.tolist()))
    if key not in _CACHE:
        _CACHE[key] = (_build_launch1(TA, TB), _build_launch2(TA, TB))
    nc1, nc2 = _CACHE[key]

    wl1 = np.einsum("khd,hd->kh", W1.reshape(128, H1, D1), al1)
    wr1 = np.einsum("khd,hd->kh", W1.reshape(128, H1, D1), ar1)
    w1aug = np.concatenate([W1, wl1, wr1], axis=1).astype(np.float32)
    wl2 = np.einsum("khd,hd->kh", W2.reshape(256, 1, 32), al2)
    wr2 = np.einsum("khd,hd->kh", W2.reshape(256, 1, 32), ar2)
    w2a = np.concatenate([W2, wl2, wr2], axis=1).astype(np.float32)
    w2aug = np.concatenate([w2a[:P], w2a[P:]], axis=1)

    x_perm = np.zeros((NPAD, 128), np.float32)
    x_perm[newid] = inputs
    xt_all = np.ascontiguousarray(
        x_perm.reshape(GBLOCKS, P, 128).transpose(0, 2, 1)
    )
    identity = np.eye(P, dtype=np.float32)

    in_maps1 = []
    for c in range(NCORES):
        in_maps1.append({
            "xt": xt_all, "w1aug": w1aug, "w2aug": w2aug,
            "identin": identity,
            "idxain": np.ascontiguousarray(prep["idxA"][c]),
            "idxbin": np.ascontiguousarray(prep["idxB"][c]),
            "ertin": prep["ert"][c],
        })
    res1 = _run(nc1, in_maps1, "l1")

    f2_by_newid = np.concatenate(
        [res1.results[c]["f2out"] for c in range(NCORES)], axis=0
    )  # [NPAD, 34]
    tab2 = np.zeros((NTAB, ROW2), np.float32)
    tab2[_new_row(np.arange(NPAD)), 0:34] = f2_by_newid
    tab2[SENT_A, 32] = SENT_EL
    tab2[SENT_B, 32] = SENT_EL
    in_maps2 = []
    for c in range(NCORES):
        own = tab2[_new_row(np.arange(c * NPC, (c + 1) * NPC)), 33]
        er2 = np.ascontiguousarray(own.reshape(TILES, P).T.astype(np.float32))
        in_maps2.append({
            "table2": tab2,
            "idxain": np.ascontiguousarray(prep["idxA"][c]),
            "idxbin": np.ascontiguousarray(prep["idxB"][c]),
            "er2in": er2,
            "identin": identity,
        })
    res2 = _run(nc2, in_maps2, "l2")

    out_by_newid = np.concatenate(
        [res2.results[c]["outbuf"] for c in range(NCORES)], axis=0
    )
    return np.ascontiguousarray(out_by_newid[newid]).astype(np.float32)


# revision 13
# speedup vs baseline: 1.4973x; 1.1528x over previous
"""2-layer GAT on 8 Trainium2 NeuronCores (Bass/Tile).

Sharding: nodes sorted by in-degree, snake-dealt across 8 cores (6250 ->
padded 6272 per core), tiled 128/tile (49 tiles/core); partition j of tile t
owns one dst node, its incoming edges occupy slots (chunk c, partition j).
Per-core HBM node table row = [f(256)|el(8)] f32 from the projection matmul
x @ [W1|W1.al1|W1.ar1]; per-edge rows fetched by indirect-DMA gather (128
rows/chunk). alpha = exp(leaky_relu(el[src]+er[dst])) (no max-subtraction:
logits are small, softmax is shift-invariant); alpha overwrites the el
column so one identity-matmul per chunk accumulates [sum(alpha*f)|sum(alpha)]
in PSUM; divide, ELU. Layer-2 projection h1 @ [W2|wl2|wr2] per tile; host
assembles the full 34-float-row layer-2 table for launch 2 (same grids).
Padding slots point at a sentinel row (f=0, el=-300 -> alpha ~ 0).
"""
import sys

sys.path.insert(0, "/opt/trn_rl_repo")

import numpy as np

import concourse.bass as bass
import concourse.bacc as bacc
import concourse.tile as tile
from concourse import mybir
from concourse.bass_utils import run_bass_kernel_spmd

N = 50000
E = 800000
P = 128
NCORES = 8
TILES = 49                       # tiles per core
NPC = TILES * P                  # 6272 nodes per core
NPAD = NCORES * NPC              # 50176
GBLOCKS = NPAD // P              # 392 projection blocks
SPLIT_ROW = 25088                # sentinel A position
NTAB = NPAD + 2                  # 50178 table rows (two sentinels)
SENT_A = SPLIT_ROW
SENT_B = NTAB - 1
ROW1 = 280                       # bf16 [f 256 | alpha 8 | el-f32 @132]
EL1_F32 = 132                    # f32-view col of el
ROW2 = 34                        # [f2 32 | el2 1 | er2 1]
H1, D1 = 8, 32
NEG_SLOPE = 0.2
SENT_EL = -300.0
F32 = mybir.dt.float32
BF16 = mybir.dt.bfloat16
I32 = mybir.dt.int32


def _new_row(r):
    return r + (r >= SPLIT_ROW)


def _ap(t, off, dims):
    s = t[:] if not isinstance(t, bass.AP) else t
    return bass.AP(tensor=s.tensor, offset=s.offset + off, ap=[s.ap[0]] + dims)


# ----------------------------------------------------------------------------
# host preprocessing
# ----------------------------------------------------------------------------

def _prep(src, dst):
    deg = np.bincount(dst, minlength=N)
    order = np.argsort(-deg, kind="stable")
    pat = np.concatenate([np.arange(NCORES), np.arange(NCORES - 1, -1, -1)])
    core_of_pos = pat[np.arange(N) % (2 * NCORES)]
    newid = np.empty(N, np.int64)
    for c in range(NCORES):
        nodes_c = order[core_of_pos == c]
        newid[nodes_c] = c * NPC + np.arange(len(nodes_c))

    nd = newid[dst]
    ns = newid[src]

    o = np.argsort(nd, kind="stable")
    nd_s, ns_s = nd[o], ns[o]
    first = np.searchsorted(nd_s, np.arange(NPAD), side="left")
    k_s = np.arange(E) - first[nd_s]

    degn = np.bincount(nd, minlength=NPAD).reshape(NCORES, TILES, P)
    T = degn.max(axis=(0, 2)).clip(min=1).astype(np.int64)   # [TILES]
    offs = np.concatenate([[0], np.cumsum(T)])
    TS = int(offs[-1])

    # per-core block order: own 49 blocks first, then the rest
    blockpos = np.empty((NCORES, GBLOCKS), np.int64)
    xt_order = np.empty((NCORES, GBLOCKS), np.int64)
    for c in range(NCORES):
        own = np.arange(c * TILES, (c + 1) * TILES)
        rest = np.concatenate(
            [np.arange(0, c * TILES), np.arange((c + 1) * TILES, GBLOCKS)]
        )
        bo = np.concatenate([own, rest])
        xt_order[c] = bo
        blockpos[c][bo] = np.arange(GBLOCKS)

    # gather indices (per-core table rows of edge srcs), [NCORES, P, TS]
    idxs = np.full((NCORES, P, TS), SENT_B, np.int32)
    c_s = nd_s // NPC
    t_s = (nd_s % NPC) // P
    j_s = nd_s % P
    slot_s = offs[t_s] + k_s
    rowpos = blockpos[c_s, ns_s // P] * P + (ns_s % P)
    idxs[c_s, j_s, slot_s] = _new_row(rowpos).astype(np.int32)

    return {"newid": newid, "T": T, "idxs": idxs,
            "xt_order": xt_order, "blockpos": blockpos}


# ----------------------------------------------------------------------------
# launch 1: projection + layer-1 edges + layer-2 projection
# ----------------------------------------------------------------------------

def _build_launch1(T):
    TS = int(T.sum())
    Tmax = int(T.max())
    offs = np.concatenate([[0], np.cumsum(T)])
    nc = bacc.Bacc("TRN2", target_bir_lowering=False, debug=False,
                   num_devices=NCORES)
    xt = nc.dram_tensor("xt", [GBLOCKS, P, P], F32, kind="ExternalInput")
    w1aug = nc.dram_tensor("w1aug", [P, 272], F32, kind="ExternalInput")
    w2aug = nc.dram_tensor("w2aug", [P, 2 * ROW2], F32, kind="ExternalInput")
    identin = nc.dram_tensor("identin", [P, P], F32, kind="ExternalInput")
    idxin = nc.dram_tensor("idxin", [P, TS], I32, kind="ExternalInput")
    f2out = nc.dram_tensor("f2out", [NPC, ROW2], F32, kind="ExternalOutput")
    dbg = nc.dram_tensor("dbg", [P, ROW1], BF16, kind="ExternalOutput")
    table = nc.dram_tensor("table", [NTAB, ROW1], BF16, kind="Internal")

    er_sb = nc.alloc_sbuf_tensor("er_sb", [P, TILES * H1], F32).ap()
    idx_sb = nc.alloc_sbuf_tensor("idx_sb", [P, TS], I32).ap()
    ident_sb = nc.alloc_sbuf_tensor("ident_sb", [P, P], F32).ap()
    ident16 = nc.alloc_sbuf_tensor("ident16", [P, P], BF16).ap()
    w2_sb = nc.alloc_sbuf_tensor("w2_sb", [P, 2 * ROW2], F32).ap()

    # ---- phase 1: projection builds the node table --------------------------
    with tile.TileContext(nc) as tc:
        with (
            tc.tile_pool(name="p1sbuf", bufs=3) as pool,
            tc.tile_pool(name="p1psum", bufs=4, space="PSUM") as psum,
            tc.tile_pool(name="p1const", bufs=1) as consts,
        ):
            w1_sb = consts.tile([P, 272], F32)
            nc.sync.dma_start(out=w1_sb[:], in_=w1aug[:])
            nc.sync.dma_start(out=ident_sb, in_=identin[:])
            nc.vector.tensor_copy(out=ident16, in_=ident_sb)
            nc.sync.dma_start(out=w2_sb, in_=w2aug[:])
            nc.sync.dma_start(out=idx_sb, in_=idxin[:])
            sent_sb = consts.tile([1, ROW1], BF16)
            nc.vector.memset(sent_sb[:], 0.0)
            nc.vector.memset(
                sent_sb[:].bitcast(F32)[:, EL1_F32:EL1_F32 + H1], SENT_EL)
            nc.sync.dma_start(out=table[SENT_A:SENT_A + 1, :], in_=sent_sb[:])
            nc.sync.dma_start(out=table[SENT_B:SENT_B + 1, :], in_=sent_sb[:])
            for b in range(GBLOCKS):
                xtile = pool.tile([P, P], F32, tag="xt")
                nc.sync.dma_start(out=xtile[:], in_=xt[b])
                pp = psum.tile([P, 272], F32, tag="pp")
                nc.tensor.matmul(pp[:], xtile[:], w1_sb[:],
                                 start=True, stop=True)
                fo = pool.tile([P, ROW1], BF16, tag="fo")
                nc.scalar.activation(out=fo[:, 0:256], in_=pp[:, 0:256],
                                     func=mybir.ActivationFunctionType.Copy)
                nc.vector.tensor_copy(
                    out=fo[:].bitcast(F32)[:, EL1_F32:EL1_F32 + H1],
                    in_=pp[:, 256:264])
                if b < TILES:
                    nc.vector.tensor_copy(
                        out=er_sb[:, b * H1:(b + 1) * H1], in_=pp[:, 264:272]
                    )
                r0 = int(_new_row(b * P))
                nc.sync.dma_start(out=table[r0:r0 + P, :], in_=fo[:])

    with tile.TileContext(nc) as tc:
        with tc.tile_pool(name="dbgp", bufs=1) as dp:
            dt_ = dp.tile([P, ROW1], BF16)
            nc.sync.dma_start(out=dt_[:], in_=table[0:P, :])
            nc.sync.dma_start(out=dbg[:], in_=dt_[:])

    # ---- phase 2: layer-1 edges + layer-2 projection ------------------------
    with tile.TileContext(nc) as tc:
        with (
            tc.tile_pool(name="p2sbuf", bufs=3) as pool,
            tc.tile_pool(name="p2small", bufs=3) as small,
            tc.tile_pool(name="p2psum", bufs=3, space="PSUM") as psum,
            tc.tile_pool(name="p2psumT", bufs=2, space="PSUM") as psumT,
            tc.tile_pool(name="p2psum2", bufs=2, space="PSUM") as psum2,
        ):
            for t in range(TILES):
                Tt = int(T[t])
                o0 = int(offs[t])
                g = pool.tile([P, Tmax * ROW1], BF16, tag="g")
                gs = g[:]
                gv = gs.rearrange("p (c f) -> p c f", f=ROW1)
                for c in range(Tt):
                    nc.gpsimd.indirect_dma_start(
                        out=gv[:, c, :],
                        out_offset=None,
                        in_=table[:],
                        in_offset=bass.IndirectOffsetOnAxis(
                            ap=idx_sb[:, o0 + c:o0 + c + 1], axis=0
                        ),
                    )
                # logits lt = el[src] + er[dst]   [P, Tt*8]
                lt = small.tile([P, Tmax * H1], F32, tag="lt")
                el_ap = _ap(gs.bitcast(F32), EL1_F32,
                            [[ROW1 // 2, Tt], [1, H1]])
                er_ap = _ap(er_sb, t * H1, [[0, Tt], [1, H1]])
                lt_ap = _ap(lt, 0, [[H1, Tt], [1, H1]])
                nc.vector.tensor_tensor(out=lt_ap, in0=el_ap, in1=er_ap,
                                        op=mybir.AluOpType.add)
                # leaky relu: lt = max(lt, 0.2*lt)
                lt2 = small.tile([P, Tmax * H1], F32, tag="lt2")
                nc.vector.tensor_scalar_mul(lt2[:, :Tt * H1],
                                            lt[:, :Tt * H1], NEG_SLOPE)
                nc.vector.tensor_tensor(out=lt[:, :Tt * H1],
                                        in0=lt[:, :Tt * H1],
                                        in1=lt2[:, :Tt * H1],
                                        op=mybir.AluOpType.max)
                # alpha = exp(lt) -> el column of g
                al_ap = _ap(gs, 256, [[ROW1, Tt], [1, H1]])
                nc.scalar.activation(out=al_ap, in_=lt_ap,
                                     func=mybir.ActivationFunctionType.Exp)
                # msg scale: g[:, :, 0:256] *= alpha (broadcast over d)
                f_ap = _ap(gs, 0, [[ROW1, Tt], [32, H1], [1, 32]])
                ab_ap = _ap(gs, 256, [[ROW1, Tt], [1, H1], [0, 32]])
                nc.vector.tensor_tensor(out=f_ap, in0=f_ap, in1=ab_ap,
                                        op=mybir.AluOpType.mult)
                # aggregate: acc = [sum alpha*f | sum alpha]
                acc = psum.tile([P, 264], F32, tag="acc")
                for c in range(Tt):
                    nc.tensor.matmul(acc[:], ident16, gv[:, c, 0:264],
                                     start=(c == 0), stop=(c == Tt - 1))
                # h1 = elu(acc[:, :256] / denom)   (b1 == 0)
                rec = small.tile([P, H1], F32, tag="rec")
                nc.vector.reciprocal(rec[:], acc[:, 256:264])
                h1 = pool.tile([P, 256], F32, tag="h1")
                acc_f = _ap(acc, 0, [[32, H1], [1, 32]])
                rb_ap = _ap(rec, 0, [[1, H1], [0, 32]])
                h1_ap = _ap(h1, 0, [[32, H1], [1, 32]])
                nc.vector.tensor_tensor(out=h1_ap, in0=acc_f, in1=rb_ap,
                                        op=mybir.AluOpType.mult)
                # ELU: h1 = max(h1, exp(min(h1,0)) - 1)
                e1 = pool.tile([P, 256], F32, tag="e1")
                nc.vector.tensor_scalar_min(e1[:], h1[:], 0.0)
                nc.scalar.activation(out=e1[:], in_=e1[:],
                                     func=mybir.ActivationFunctionType.Exp)
                nc.vector.tensor_scalar_add(e1[:], e1[:], -1.0)
                nc.vector.tensor_tensor(out=h1[:], in0=h1[:], in1=e1[:],
                                        op=mybir.AluOpType.max)
                # layer-2 projection: f2 = h1 @ w2aug
                f2p = psum2.tile([P, ROW2], F32, tag="f2p")
                for k in range(2):
                    tp = psumT.tile([P, P], F32, tag="tp")
                    nc.tensor.transpose(out=tp[:],
                                        in_=h1[:, k * P:(k + 1) * P],
                                        identity=ident_sb)
                    h1t = small.tile([P, P], F32, tag="h1t")
                    nc.vector.tensor_copy(out=h1t[:], in_=tp[:])
                    nc.tensor.matmul(f2p[:], h1t[:],
                                     w2_sb[:, k * ROW2:(k + 1) * ROW2],
                                     start=(k == 0), stop=(k == 1))
                f2s = small.tile([P, ROW2], F32, tag="f2s")
                nc.scalar.activation(out=f2s[:], in_=f2p[:],
                                     func=mybir.ActivationFunctionType.Copy)
                nc.sync.dma_start(out=f2out[t * P:(t + 1) * P, :], in_=f2s[:])
    nc.compile()
    return nc


# ----------------------------------------------------------------------------
# launch 2: layer-2 edge aggregation
# ----------------------------------------------------------------------------

def _build_launch2(T):
    TS = int(T.sum())
    Tmax = int(T.max())
    offs = np.concatenate([[0], np.cumsum(T)])
    nc = bacc.Bacc("TRN2", target_bir_lowering=False, debug=False,
                   num_devices=NCORES)
    table2 = nc.dram_tensor("table2", [NTAB, ROW2], F32, kind="ExternalInput")
    idxin = nc.dram_tensor("idxin", [P, TS], I32, kind="ExternalInput")
    er2in = nc.dram_tensor("er2in", [P, TILES], F32, kind="ExternalInput")
    identin = nc.dram_tensor("identin", [P, P], F32, kind="ExternalInput")
    outbuf = nc.dram_tensor("outbuf", [NPC, 32], F32, kind="ExternalOutput")

    with tile.TileContext(nc) as tc:
        with (
            tc.tile_pool(name="l2sbuf", bufs=2) as pool,
            tc.tile_pool(name="l2small", bufs=3) as small,
            tc.tile_pool(name="l2psum", bufs=3, space="PSUM") as psum,
            tc.tile_pool(name="l2const", bufs=1) as consts,
        ):
            ident_sb = consts.tile([P, P], F32)
            nc.sync.dma_start(out=ident_sb[:], in_=identin[:])
            idx_sb = consts.tile([P, TS], I32)
            nc.sync.dma_start(out=idx_sb[:], in_=idxin[:])
            er2_sb = consts.tile([P, TILES], F32)
            nc.sync.dma_start(out=er2_sb[:], in_=er2in[:])
            for t in range(TILES):
                Tt = int(T[t])
                o0 = int(offs[t])
                g = pool.tile([P, Tmax * ROW2], F32, tag="g")
                gs = g[:]
                gv = gs.rearrange("p (c f) -> p c f", f=ROW2)
                for c in range(Tt):
                    nc.gpsimd.indirect_dma_start(
                        out=gv[:, c, :],
                        out_offset=None,
                        in_=table2[:],
                        in_offset=bass.IndirectOffsetOnAxis(
                            ap=idx_sb[:, o0 + c:o0 + c + 1], axis=0
                        ),
                    )
                lt = small.tile([P, Tmax], F32, tag="lt")
                el_ap = _ap(gs, 32, [[ROW2, Tt]])
                er_ap = _ap(er2_sb, t, [[0, Tt]])
                nc.vector.tensor_tensor(out=lt[:, :Tt], in0=el_ap, in1=er_ap,
                                        op=mybir.AluOpType.add)
                lt2 = small.tile([P, Tmax], F32, tag="lt2")
                nc.vector.tensor_scalar_mul(lt2[:, :Tt], lt[:, :Tt], NEG_SLOPE)
                nc.vector.tensor_tensor(out=lt[:, :Tt], in0=lt[:, :Tt],
                                        in1=lt2[:, :Tt],
                                        op=mybir.AluOpType.max)
                al_ap = _ap(gs, 32, [[ROW2, Tt]])
                nc.scalar.activation(out=al_ap, in_=lt[:, :Tt],
                                     func=mybir.ActivationFunctionType.Exp)
                f_ap = _ap(gs, 0, [[ROW2, Tt], [1, 32]])
                ab_ap = _ap(gs, 32, [[ROW2, Tt], [0, 32]])
                nc.vector.tensor_tensor(out=f_ap, in0=f_ap, in1=ab_ap,
                                        op=mybir.AluOpType.mult)
                acc = psum.tile([P, 33], F32, tag="acc")
                for c in range(Tt):
                    nc.tensor.matmul(acc[:], ident_sb[:], gv[:, c, 0:33],
                                     start=(c == 0), stop=(c == Tt - 1))
                rec = small.tile([P, 1], F32, tag="rec")
                nc.vector.reciprocal(rec[:], acc[:, 32:33])
                o2 = small.tile([P, 32], F32, tag="o2")
                nc.vector.tensor_scalar_mul(o2[:], acc[:, 0:32], rec[:, 0:1])
                nc.sync.dma_start(out=outbuf[t * P:(t + 1) * P, :], in_=o2[:])
    nc.compile()
    return nc


# ----------------------------------------------------------------------------
# entry point
# ----------------------------------------------------------------------------

_CACHE = {}
PROFILE = False
LAST_EXEC_NS = []


def _run(nc, in_maps, tag):
    if PROFILE:
        import tempfile
        res = run_bass_kernel_spmd(
            nc, in_maps, core_ids=list(range(NCORES)), trace=True,
            tmpdir=tempfile.mkdtemp(prefix=f"gat_{tag}_"),
        )
        LAST_EXEC_NS.append((tag, res.exec_time_ns))
        return res
    return run_bass_kernel_spmd(nc, in_maps, core_ids=list(range(NCORES)))
PROFILE = False
LAST_EXEC_NS = []


def _run(nc, in_maps, tag):
    if PROFILE:
        import tempfile
        res = run_bass_kernel_spmd(
            nc, in_maps, core_ids=list(range(NCORES)), trace=True,
            tmpdir=tempfile.mkdtemp(prefix=f"gat_{tag}_"),
        )
        LAST_EXEC_NS.append((tag, res.exec_time_ns))
        return res
    return run_bass_kernel_spmd(nc, in_maps, core_ids=list(range(NCORES)))


def kernel(inputs, src, dst, W1, al1, ar1, b1, W2, al2, ar2, b2):
    inputs = np.asarray(inputs, np.float32)
    src = np.asarray(src).astype(np.int64)
    dst = np.asarray(dst).astype(np.int64)
    W1 = np.asarray(W1, np.float32)
    W2 = np.asarray(W2, np.float32)
    al1 = np.asarray(al1, np.float32)
    ar1 = np.asarray(ar1, np.float32)
    al2 = np.asarray(al2, np.float32)
    ar2 = np.asarray(ar2, np.float32)

    prep = _prep(src, dst)
    T, idxs = prep["T"], prep["idxs"]
    newid, xt_order = prep["newid"], prep["xt_order"]

    key = tuple(T.tolist())
    if key not in _CACHE:
        _CACHE[key] = (_build_launch1(T), _build_launch2(T))
    nc1, nc2 = _CACHE[key]

    wl1 = np.einsum("khd,hd->kh", W1.reshape(128, H1, D1), al1)
    wr1 = np.einsum("khd,hd->kh", W1.reshape(128, H1, D1), ar1)
    w1aug = np.concatenate([W1, wl1, wr1], axis=1).astype(np.float32)
    wl2 = np.einsum("khd,hd->kh", W2.reshape(256, 1, 32), al2)
    wr2 = np.einsum("khd,hd->kh", W2.reshape(256, 1, 32), ar2)
    w2a = np.concatenate([W2, wl2, wr2], axis=1).astype(np.float32)  # [256,34]
    w2aug = np.concatenate([w2a[:P], w2a[P:]], axis=1)               # [128,68]

    x_perm = np.zeros((NPAD, 128), np.float32)
    x_perm[newid] = inputs
    identity = np.eye(P, dtype=np.float32)

    in_maps1 = []
    for c in range(NCORES):
        xt_c = np.ascontiguousarray(
            x_perm.reshape(GBLOCKS, P, 128)[xt_order[c]].transpose(0, 2, 1)
        )
        in_maps1.append({
            "xt": xt_c, "w1aug": w1aug, "w2aug": w2aug,
            "identin": identity,
            "idxin": np.ascontiguousarray(idxs[c]),
        })
    res1 = _run(nc1, in_maps1, "l1")

    f2_by_newid = np.concatenate(
        [res1.results[c]["f2out"] for c in range(NCORES)], axis=0
    ).reshape(GBLOCKS, P, ROW2)
    in_maps2 = []
    for c in range(NCORES):
        tab2 = np.zeros((NTAB, ROW2), np.float32)
        rows = _new_row(prep["blockpos"][c] * P)
        for gblk in range(GBLOCKS):
            tab2[rows[gblk]:rows[gblk] + P] = f2_by_newid[gblk]
        tab2[SENT_A, 32] = SENT_EL
        tab2[SENT_B, 32] = SENT_EL
        er2 = np.ascontiguousarray(tab2[:NPC, 33].reshape(TILES, P).T)
        in_maps2.append({
            "table2": tab2,
            "idxin": np.ascontiguousarray(idxs[c]),
            "er2in": er2,
            "identin": identity,
        })
    res2 = _run(nc2, in_maps2, "l2")

    out_by_newid = np.concatenate(
        [res2.results[c]["outbuf"] for c in range(NCORES)], axis=0
    )
    return np.ascontiguousarray(out_by_newid[newid]).astype(np.float32)
